# revision 39
# baseline (speedup 1.0000x reference)
"""AgentAttention fused Trainium2 kernel (8-core data-parallel over batch).

Reference computation (per batch, n=3136=56x56, c=384, 8 heads, 16 agents):
    q = x @ Wq.T ; k = x @ Wkv.T
    agent_q = pool(q); agent_k = pool(k)            # 4x4 adaptive avg pool
    A = (agent_q @ w_g) * scale; G = sum(A * agent_q, 1)
    agent_new = (G * agent_k) @ Wp.T + bp + agent_q
    attn = softmax(scale * q_h @ agent_new_h.T)     # per head
    out = (attn @ agent_k_h) -> concat -> @ Wp.T + bp

Key algebraic fusions (same math as v1):
  1. pooling commutes with the linear projections, so full q/k are never
     computed: agent_q = pool(x) @ Wq.T etc.
  2. logitsT = mbdT.T @ xT where mbdT folds Wq into the block-diagonal
     agent_new layout.
  3. softmax without max subtraction (logits are O(0.1)).
  4. per-head softmax sums via a 0/1 head-selector matmul; 1/sum
     broadcast back to 128 partitions via a second tiny matmul.
  5. out projection folded into the value matrix akp = akbd.T @ Wp.T
     (+ bp/8 via a rank-1 matmul, since softmax rows sum to 1 per head).

v4 performance notes (v1 198us, v2 181us, v3 180us):
  - tokens are PERMUTED on the host into pooling-block-major order
    ((hi,wi,hr,wr) instead of raster), so each 14x14 pooling block is
    196 CONTIGUOUS columns: pooling becomes ONE stride-1 TensorReduce
    per (group, ci) slab and is eligible for the DVE 16-bit fast path.
    Attention is token-order agnostic; the host un-permutes the output.
  - x transposed on the HOST into [b, ci, 128, n'] bf16 slabs; loads
    split across BOTH HWDGE rings (one ring sustains only ~233 GB/s).
  - output produced TRANSPOSED in bf16 (3 matmuls/chunk), buffered per
    batch in SBUF, stored as 3 contiguous DMAs per batch.
  - ALL chunk-loop matmul operands bf16: fp32/f32r moving operands run
    2-3x slower; the fp32 softmax-broadcast matmul was compiler-split
    into a ~2us LOW_HIGH pair per chunk (reciprocals now cast to bf16
    on ACT before the broadcast matmul).
  - elementwise work (exp, reciprocal, normalize-mult, PSUM->SBUF bf16
    casts) is spread across DVE / ACT / GPSIMD.
  - tile_wait_until floors on injected agent groups keep the greedy
    Tile scheduler from front-running big pool reduces into DVE idle
    gaps ahead of the critical agent chain.
"""

import numpy as np
import ml_dtypes
from contextlib import ExitStack, nullcontext

import concourse.bass as bass
import concourse.bacc as bacc
import concourse.mybir as mybir
import concourse.tile as tile
from concourse.bass_utils import run_bass_kernel_spmd

NCORES = 8
B_FULL = 32
BPC = B_FULL // NCORES   # 4 batches per core
N = 3136                 # 56*56
C = 384
P = 128
CH = C // P              # 3 c-chunks
NCHUNK = 448
NCH = N // NCHUNK        # 7 n-chunks per batch
HEADS = 8
D = C // HEADS           # 48
A = 16                   # agents
HA = HEADS * A           # 128
SCALE = float(D) ** -0.5
POOLN = 196.0            # 14*14 elements per pooling block

F32 = mybir.dt.float32
BF16 = mybir.dt.bfloat16

ADD = mybir.AluOpType.add
MULT = mybir.AluOpType.mult

# constant blob column layouts (per 128-partition row, in elements).
# blob16a holds only what the batch-0 agent chain needs first, so it
# can load before batch 0's x slab on the ACT ring; blob16b follows.
_F32_SECTS = [("bpv", CH), ("bdmask", CH * HA)]
_B16A_SECTS = [("wqt16", CH * C), ("wkvt16", CH * C), ("wg16", CH),
               ("ones16", HA)]
_B16B_SECTS = [("wq16", CH * C), ("wpt16", CH * C),
               ("hsel40", 40), ("hselt16", HA), ("bp816", C)]
F32_COLS = sum(n for _, n in _F32_SECTS)
B16A_COLS = sum(n for _, n in _B16A_SECTS)
B16B_COLS = sum(n for _, n in _B16B_SECTS)


def _offsets(sects):
    out, o = {}, 0
    for name, n in sects:
        out[name] = (o, n)
        o += n
    return out


F32_OFF = _offsets(_F32_SECTS)
B16A_OFF = _offsets(_B16A_SECTS)
B16B_OFF = _offsets(_B16B_SECTS)

# agent-phase batch groups + scheduler floors (ms of simulated time;
# keeps each group's pool reduces out of the DVE queue until its x
# slabs have landed).
GROUPS = [[0], [1], [2], [3]]
G_FLOOR = [None, 0.024, 0.034, 0.044]


def build_nc(stage=None):
    nc = bacc.Bacc(None, target_bir_lowering=False, debug=False)

    x16t = nc.dram_tensor("x16t", [BPC, CH, P, N], BF16,
                          kind="ExternalInput")
    blob16a = nc.dram_tensor("blob16a", [P, B16A_COLS], BF16,
                             kind="ExternalInput")
    blob16b = nc.dram_tensor("blob16b", [P, B16B_COLS], BF16,
                             kind="ExternalInput")
    blobf = nc.dram_tensor("blobf", [P, F32_COLS], F32, kind="ExternalInput")
    out16t = nc.dram_tensor("out16t", [BPC, CH, P, N], BF16,
                            kind="ExternalOutput")

    with tile.TileContext(nc) as tc, ExitStack() as ctx:
        consts = ctx.enter_context(tc.tile_pool(name="consts", bufs=1))
        xtp = ctx.enter_context(tc.tile_pool(name="xt", bufs=1))
        agents = ctx.enter_context(tc.tile_pool(name="agents", bufs=3))
        akpp = ctx.enter_context(tc.tile_pool(name="akpp", bufs=3))
        chunkp = ctx.enter_context(tc.tile_pool(name="chunk", bufs=4))
        obuf = ctx.enter_context(tc.tile_pool(name="obuf", bufs=2))
        psA = ctx.enter_context(
            tc.tile_pool(name="psA", bufs=1, space=bass.MemorySpace.PSUM))
        psL = ctx.enter_context(
            tc.tile_pool(name="psL", bufs=2, space=bass.MemorySpace.PSUM))
        psS = ctx.enter_context(
            tc.tile_pool(name="psS", bufs=1, space=bass.MemorySpace.PSUM))
        psB = ctx.enter_context(
            tc.tile_pool(name="psB", bufs=1, space=bass.MemorySpace.PSUM))
        psO = ctx.enter_context(
            tc.tile_pool(name="psO", bufs=3, space=bass.MemorySpace.PSUM))

        # ---- x loads split across both HWDGE rings (b0 slabs first on
        # each so batch 0 completes ~7us in). SP: c0+c1 slabs; ACT: c2
        # slabs interleaved with the const blobs.
        xt_all = xtp.tile([P, CH, BPC * N], BF16, tag="xt")
        sb16a = consts.tile([P, B16A_COLS], BF16, tag="sb16a")
        sb16b = consts.tile([P, B16B_COLS], BF16, tag="sb16b")
        sbf = consts.tile([P, F32_COLS], F32, tag="sbf")

        def ld(b, ci, eng):
            eng.dma_start(xt_all[:, ci, b * N:(b + 1) * N], x16t[b, ci])

        nc.scalar.dma_start(sb16a[:], blob16a[:])
        ld(0, 0, nc.sync)
        ld(0, 1, nc.sync)
        ld(0, 2, nc.scalar)
        nc.scalar.dma_start(sb16b[:], blob16b[:])
        nc.scalar.dma_start(sbf[:], blobf[:])
        ld(1, 0, nc.sync)
        ld(1, 1, nc.sync)
        ld(1, 2, nc.scalar)
        ld(2, 0, nc.sync)
        ld(2, 1, nc.sync)
        ld(2, 2, nc.scalar)
        ld(3, 0, nc.sync)
        ld(3, 1, nc.sync)
        ld(3, 2, nc.scalar)

        def fview(name, nmid):
            o, n = F32_OFF[name]
            v = sbf[:, o:o + n]
            return v.rearrange("p (a b) -> p a b", a=nmid) if nmid else v

        def bview(name, nmid):
            if name in B16A_OFF:
                o, n = B16A_OFF[name]
                v = sb16a[:, o:o + n]
            else:
                o, n = B16B_OFF[name]
                v = sb16b[:, o:o + n]
            return v.rearrange("p (a b) -> p a b", a=nmid) if nmid else v

        wq_sb = bview("wq16", CH)
        wqt_sb = bview("wqt16", CH)
        wkvt_sb = bview("wkvt16", CH)
        wpt16_sb = bview("wpt16", CH)
        wg_sb = bview("wg16", CH)
        ones16_sb = bview("ones16", 0)[0:1, :]
        hsel40_sb = bview("hsel40", 0)
        hsel16_sb = hsel40_sb[:, 0:HEADS]
        hselt16_v = bview("hselt16", 0)
        hselt16_sb = [hselt16_v[0:HEADS, :],
                      hselt16_v[32:32 + HEADS, :]]
        bp816_sb = bview("bp816", 0)[0:1, :]
        bp_sb = fview("bpv", CH)
        bdm_sb = fview("bdmask", CH)

        bstate = {}

        def agent_steps(bs):
            """Agent phase for a group of batches (pool -> projections ->
            gating -> agent_new -> mbdT/akp), batched along the free axis
            so the many small matmuls get L*16 columns instead of 16."""
            L = len(bs)
            k0 = bs[0]
            AG = L * A
            st = {}

            def pool(ci):
                def f():
                    if ci == 0:
                        st["xpT"] = agents.tile([P, CH, AG], BF16,
                                                tag="xpT", name="xpT")
                    # block-major token order: contiguous stride-1
                    # reduces, split into 4-agent pieces (~0.9us) so
                    # they never head-of-line-block the DVE queue
                    v = xt_all[:, ci, k0 * N:(k0 + L) * N].rearrange(
                        "p (a r) -> p a r", a=L * A)
                    xp = st["xpT"][:, ci, :].rearrange(
                        "p (a x) -> p a x", a=L * A)
                    with nc.allow_low_precision(reason="bf16 pooling"):
                        for a0 in range(0, L * A, 4):
                            nc.vector.tensor_reduce(
                                xp[:, a0:a0 + 4, :], v[:, a0:a0 + 4, :],
                                axis=mybir.AxisListType.X, op=ADD)
                return f

            def proj(dst_name, w_sb):
                def f():
                    ps = psA.tile([P, CH, AG], F32, tag="ag", name="agps")
                    for co in range(CH):
                        for ci in range(CH):
                            nc.tensor.matmul(
                                ps[:, co, :], w_sb[:, ci, co * P:(co + 1) * P],
                                st["xpT"][:, ci, :],
                                start=(ci == 0), stop=(ci == CH - 1))
                    t = agents.tile([P, CH, AG], BF16, tag=dst_name,
                                    name=dst_name)
                    nc.scalar.copy(t[:], ps[:])
                    st[dst_name] = t
                return f

            def gate1():
                a_ps = psA.tile([1, AG], F32, tag="ag", name="agps")
                for ci in range(CH):
                    nc.tensor.matmul(a_ps[:], wg_sb[:, ci, :],
                                     st["aqT"][:, ci, :],
                                     start=(ci == 0), stop=(ci == CH - 1))
                a_sb = agents.tile([1, AG], BF16, tag="a_sb", name="a_sb")
                nc.scalar.copy(a_sb[:], a_ps[:])  # SCALE folded into wg16
                ar_ps = psA.tile([P, AG], F32, tag="ag", name="agps")
                nc.tensor.matmul(ar_ps[:], ones16_sb[:], a_sb[:],
                                 start=True, stop=True)
                st["ar_ps"] = ar_ps

            def gate2():
                gscr = agents.tile([P, CH, AG], F32, tag="gscr", name="gscr")
                for ci in range(CH):
                    nc.vector.tensor_mul(gscr[:, ci, :], st["aqT"][:, ci, :],
                                         st["ar_ps"][:])
                gvec = agents.tile([P, CH * L], F32, tag="gvec", name="gvec")
                nc.vector.tensor_reduce(
                    gvec[:].rearrange("p (cb x) -> p cb x", cb=CH * L),
                    gscr[:].rearrange("p c (b a) -> p (c b) a", b=L),
                    axis=mybir.AxisListType.X, op=ADD)
                gkT = agents.tile([P, CH, AG], BF16, tag="gkT", name="gkT")
                for ci in range(CH):
                    for j in range(L):
                        nc.scalar.mul(
                            gkT[:, ci, j * A:(j + 1) * A],
                            st["akT"][:, ci, j * A:(j + 1) * A],
                            gvec[:, ci * L + j:ci * L + j + 1])
                st["gkT"] = gkT

            def an_mms():
                an_ps = psA.tile([P, CH, AG], F32, tag="ag", name="agps")
                for co in range(CH):
                    for ci in range(CH):
                        nc.tensor.matmul(
                            an_ps[:, co, :],
                            wpt16_sb[:, ci, co * P:(co + 1) * P],
                            st["gkT"][:, ci, :],
                            start=(ci == 0), stop=(ci == CH - 1))
                st["an_ps"] = an_ps

            def bd():
                an_st = agents.tile([P, CH, AG], F32, tag="an_st",
                                    name="an_st")
                for ci in range(CH):
                    nc.vector.tensor_add(an_st[:, ci, :],
                                         st["an_ps"][:, ci, :],
                                         st["aqT"][:, ci, :])
                    nc.vector.tensor_scalar(an_st[:, ci, :], an_st[:, ci, :],
                                            bp_sb[:, ci, :], SCALE, ADD, MULT)
                anbd = agents.tile([P, CH, L * HA], BF16, tag="anbd",
                                   name="anbd")
                akbd = agents.tile([P, CH, L * HA], BF16, tag="akbd",
                                   name="akbd")
                with nc.allow_low_precision(reason="bf16 blockdiag"):
                    for ci in range(CH):
                        src_an = an_st[:, ci, :].rearrange(
                            "p (b a) -> p b a", b=L).unsqueeze(2).broadcast_to(
                            [P, L, HEADS, A])
                        src_ak = st["akT"][:, ci, :].rearrange(
                            "p (b a) -> p b a", b=L).unsqueeze(2).broadcast_to(
                            [P, L, HEADS, A])
                        msk = bdm_sb[:, ci, :].rearrange(
                            "p (h a) -> p h a",
                            h=HEADS).unsqueeze(1).broadcast_to(
                            [P, L, HEADS, A])
                        nc.vector.tensor_mul(
                            anbd[:, ci, :].rearrange("p (b h a) -> p b h a",
                                                     b=L, h=HEADS),
                            src_an, msk)
                        nc.vector.tensor_mul(
                            akbd[:, ci, :].rearrange("p (b h a) -> p b h a",
                                                     b=L, h=HEADS),
                            src_ak, msk)
                st["anbd"], st["akbd"] = anbd, akbd

            def mbd():
                mbdT = agents.tile([P, CH, L * HA], BF16, tag="mbdT",
                                   name="mbdT")
                for ci in range(CH):
                    mbd_ps = psA.tile([P, L * HA], F32, tag="ag",
                                      name="mbd_ps")
                    for co in range(CH):
                        nc.tensor.matmul(mbd_ps[:],
                                         wq_sb[:, co, ci * P:(ci + 1) * P],
                                         st["anbd"][:, co, :],
                                         start=(co == 0), stop=(co == CH - 1))
                    nc.scalar.copy(mbdT[:, ci, :], mbd_ps[:])
                st["mbdT"] = mbdT
                for j, b in enumerate(bs):
                    bstate[b] = {"mbdT": mbdT, "j": j}

            def akp(j, b):
                def f():
                    akp_ps = psA.tile([P, C], F32, tag="ag", name="akp_ps")
                    for ci in range(CH):
                        nc.tensor.matmul(
                            akp_ps[:],
                            st["akbd"][:, ci, j * HA:(j + 1) * HA],
                            wpt16_sb[:, ci, :], start=(ci == 0), stop=False)
                    nc.tensor.matmul(akp_ps[:], ones16_sb[:], bp816_sb[:],
                                     start=False, stop=True)
                    akp_sb = akpp.tile([P, C], BF16, tag="akp",
                                       name="akp_sb")
                    nc.scalar.copy(akp_sb[:], akp_ps[:])
                    bstate[b]["akp"] = akp_sb
                return f

            steps = [pool(0), pool(1), pool(2),
                     proj("aqT", wqt_sb), proj("akT", wkvt_sb),
                     gate1, gate2, an_mms, bd, mbd]
            steps += [akp(j, b) for j, b in enumerate(bs)]
            return steps

        NSLOT = BPC * NCH
        slots = [(b, t) for b in range(BPC) for t in range(NCH)]
        sstate = [dict() for _ in range(NSLOT)]

        def s0(i):
            b, t = slots[i]
            st = sstate[i]
            l_ps = psL.tile([P, NCHUNK], F32, tag="l", name="l_ps")
            mbdT, j = bstate[b]["mbdT"], bstate[b]["j"]
            for ci in range(CH):
                nc.tensor.matmul(l_ps[:],
                                 mbdT[:, ci, j * HA:(j + 1) * HA],
                                 xt_all[:, ci, b * N + t * NCHUNK:
                                        b * N + (t + 1) * NCHUNK],
                                 start=(ci == 0), stop=(ci == CH - 1))
            expT = chunkp.tile([P, NCHUNK], BF16, tag="exp", name="expT")
            with nc.allow_low_precision(reason="bf16 attn"):
                nc.scalar.activation(expT[:], l_ps[:],
                                     mybir.ActivationFunctionType.Exp)
            st["expT"] = expT

        def s1pair(p):
            # two chunks share one reciprocal + one bf16 cast: the two
            # sums matmuls write disjoint partition slices of one PSUM
            # tile, halving the per-chunk DVE/ACT normalization cost
            ii = (2 * p, 2 * p + 1)
            # matmul out base partition must be 0/32/64: pack the pair's
            # sums at partitions 0-7 and 32-39 of one PSUM tile
            s_ps = psS.tile([40, NCHUNK], F32, tag="s", name="s_ps")
            # first MM writes all 40 partitions (cols 8-39 of hsel40 are
            # zero) so the full-tile reciprocal never reads uninit PSUM
            nc.tensor.matmul(s_ps[:], hsel40_sb[:],
                             sstate[ii[0]]["expT"][:],
                             start=True, stop=True)
            nc.tensor.matmul(s_ps[32:32 + HEADS, :], hsel16_sb[:],
                             sstate[ii[1]]["expT"][:],
                             start=True, stop=True)
            rec = chunkp.tile([40, NCHUNK], F32, tag="rec", name="rec")
            nc.vector.reciprocal_approx_fast(rec[:], s_ps[:])
            rec16 = chunkp.tile([40, NCHUNK], BF16, tag="rec16",
                                name="rec16")
            nc.scalar.copy(rec16[:], rec[:])
            for k, i in enumerate(ii):
                st = sstate[i]
                b_ps = psB.tile([P, NCHUNK], F32, tag="b", name="b_ps")
                nc.tensor.matmul(b_ps[:],
                                 hselt16_sb[k],
                                 rec16[32 * k:32 * k + HEADS, :],
                                 start=True, stop=True)
                attnT = chunkp.tile([P, NCHUNK], BF16, tag="attn",
                                    name="attnT")
                with nc.allow_low_precision(reason="bf16 attn"):
                    nc.vector.tensor_mul(attnT[:], st["expT"][:], b_ps[:])
                st["attnT"] = attnT

        def s2(i):
            b, t = slots[i]
            st = sstate[i]
            attnT = st["attnT"]
            akp_sb = bstate[b]["akp"]
            if t == 0:
                bstate[b]["ob"] = obuf.tile([P, CH, N], BF16, tag="ob",
                                            name="ob")
            ob = bstate[b]["ob"]
            ns = slice(t * NCHUNK, (t + 1) * NCHUNK)
            for ci in range(CH):
                o_ps = psO.tile([P, NCHUNK], F32, tag="op", name="o_ps")
                nc.tensor.matmul(o_ps[:], akp_sb[:, ci * P:(ci + 1) * P],
                                 attnT[:], start=True, stop=True)
                if ci == 0:
                    nc.vector.tensor_copy(ob[:, ci, ns], o_ps[:])
                else:
                    nc.scalar.copy(ob[:, ci, ns], o_ps[:])
            # store per pair of chunks (and the odd 7th) so the last
            # batch's output isn't one big 2.4MB store in the tail
            if t % 2 == 1 or t == NCH - 1:
                c0 = (t - 1) * NCHUNK if t % 2 == 1 else t * NCHUNK
                cs = slice(c0, (t + 1) * NCHUNK)
                for ci in range(CH):
                    eng = nc.sync if ci == 0 else nc.scalar
                    eng.dma_start(out16t[b, ci, :, cs], ob[:, ci, cs])

        # prologue: batch 0's agent chain runs exposed.
        for f in agent_steps(GROUPS[0]):
            f()
        # inject group g's agent steps during earlier batches' chunk
        # slots, with scheduler floors so they can't run before their
        # x slabs land.
        inject = {}
        for g, (bhost, t0, t1) in ((1, (0, 1, 7)), (2, (1, 0, 7)),
                                   (3, (2, 0, 7))):
            steps = agent_steps(GROUPS[g])
            floor = G_FLOOR[g]
            nsl = t1 - t0
            for t in range(t0, t1):
                k0 = len(steps) * (t - t0) // nsl
                k1 = len(steps) * (t - t0 + 1) // nsl
                inject.setdefault((bhost, t), []).extend(
                    (floor, f) for f in steps[k0:k1])

        NP = NSLOT // 2
        for p in range(NP + 2):
            for i in (2 * p, 2 * p + 1):
                if i < NSLOT:
                    s0(i)
                    for floor, f in inject.get(slots[i], ()):
                        cmgr = (tc.tile_wait_until(floor)
                                if floor is not None else nullcontext())
                        with cmgr:
                            f()
            if 1 <= p <= NP:
                s1pair(p - 1)
            if p >= 2:
                s2(2 * p - 4)
                s2(2 * p - 3)

    nc.finalize()
    return nc


_CACHE = {}


def _get_nc():
    if "nc" not in _CACHE:
        _CACHE["nc"] = build_nc()
    return _CACHE["nc"]


def _make_const_inputs(Wq, Wkv, w_g, Wp, bp):
    Wq = np.ascontiguousarray(np.asarray(Wq, np.float32))
    Wkv = np.ascontiguousarray(np.asarray(Wkv, np.float32))
    Wp = np.ascontiguousarray(np.asarray(Wp, np.float32))
    w_g = np.ascontiguousarray(np.asarray(w_g, np.float32))
    bp = np.ascontiguousarray(np.asarray(bp, np.float32))

    def chunked(m):  # [CH*P, X] -> [P, CH*X] partition-major layout
        x = m.reshape(CH, P, -1).transpose(1, 0, 2)
        return np.ascontiguousarray(x).reshape(P, -1)

    hsel = np.zeros((HA, HEADS), np.float32)
    hsel[np.arange(HA), np.arange(HA) // A] = 1.0
    bdmask = np.zeros((C, HA), np.float32)
    for c in range(C):
        h = c // D
        bdmask[c, h * A:(h + 1) * A] = 1.0

    fparts = {
        "bpv": chunked(bp.reshape(C, 1)),
        "bdmask": chunked(bdmask),
    }
    blobf = np.concatenate([fparts[n] for n, _ in _F32_SECTS], axis=1)

    z = np.zeros((P, HA), np.float32)
    z[0] = 1.0
    zh = np.zeros((P, HA), np.float32)
    zh[:HEADS] = hsel.T
    zh[32:32 + HEADS] = hsel.T
    zb = np.zeros((P, C), np.float32)
    zb[0] = bp / HEADS
    bparts = {
        "wq16": chunked(Wq),
        "wqt16": chunked(Wq.T / POOLN),
        "wkvt16": chunked(Wkv.T / POOLN),
        "wpt16": chunked(Wp.T),
        "wg16": chunked(w_g * SCALE),
        "ones16": z,
        "hsel40": np.concatenate(
            [hsel, np.zeros((HA, 32), np.float32)], axis=1),
        "hselt16": zh,
        "bp816": zb,
    }
    blob16a = np.concatenate(
        [bparts[n] for n, _ in _B16A_SECTS],
        axis=1).astype(ml_dtypes.bfloat16)
    blob16b = np.concatenate(
        [bparts[n] for n, _ in _B16B_SECTS],
        axis=1).astype(ml_dtypes.bfloat16)
    return {"blob16a": blob16a, "blob16b": blob16b, "blobf": blobf}


def _permute_tokens(x):
    # raster (56x56) -> pooling-block-major (hi, wi, hr, wr)
    b = x.shape[0]
    return x.reshape(b, 4, 14, 4, 14, C).transpose(
        0, 1, 3, 2, 4, 5).reshape(b, N, C)


def _unpermute_tokens(y):
    # pooling-block-major -> raster
    b = y.shape[0]
    return y.reshape(b, 4, 4, 14, 14, C).transpose(
        0, 1, 3, 2, 4, 5).reshape(b, N, C)


def kernel(x, H=56, W=56, Wq=None, Wkv=None, w_g=None, Wp=None, bp=None,
           _trace=False, _trace_kwargs=None):
    x = np.asarray(x)
    assert x.shape == (B_FULL, N, C), x.shape
    x16 = np.asarray(x, np.float32).astype(ml_dtypes.bfloat16)
    # host-side: permute tokens to block-major, then transpose to
    # [b, ci, p, n'] slabs (plain DMAs on device)
    x16p = _permute_tokens(x16)
    x16t = np.ascontiguousarray(
        x16p.reshape(B_FULL, N, CH, P).transpose(0, 2, 3, 1))

    consts = _make_const_inputs(Wq, Wkv, w_g, Wp, bp)
    in_maps = []
    for c in range(NCORES):
        m = dict(consts)
        m["x16t"] = np.ascontiguousarray(x16t[c * BPC:(c + 1) * BPC])
        in_maps.append(m)

    nc = _get_nc()
    res = run_bass_kernel_spmd(nc, in_maps, list(range(NCORES)),
                               trace=_trace, **(_trace_kwargs or {}))
    # un-transpose + un-permute + upcast on host
    outs = np.concatenate(
        [_unpermute_tokens(
            np.asarray(res.results[c]["out16t"]).astype(np.float32)
            .transpose(0, 3, 1, 2).reshape(BPC, N, C))
         for c in range(NCORES)], axis=0)
    if _trace:
        return outs, res
    return outs


# revision 43
# speedup vs baseline: 1.0328x; 1.0328x over previous
"""AgentAttention fused Trainium2 kernel (8-core data-parallel over batch).

Reference computation (per batch, n=3136=56x56, c=384, 8 heads, 16 agents):
    q = x @ Wq.T ; k = x @ Wkv.T
    agent_q = pool(q); agent_k = pool(k)            # 4x4 adaptive avg pool
    A = (agent_q @ w_g) * scale; G = sum(A * agent_q, 1)
    agent_new = (G * agent_k) @ Wp.T + bp + agent_q
    attn = softmax(scale * q_h @ agent_new_h.T)     # per head
    out = (attn @ agent_k_h) -> concat -> @ Wp.T + bp

Key algebraic fusions (same math as v1):
  1. pooling commutes with the linear projections, so full q/k are never
     computed: agent_q = pool(x) @ Wq.T etc.
  2. logitsT = mbdT.T @ xT where mbdT folds Wq into the block-diagonal
     agent_new layout.
  3. softmax without max subtraction (logits are O(0.1)).
  4. per-head softmax sums via a 0/1 head-selector matmul; 1/sum
     broadcast back to 128 partitions via a second tiny matmul.
  5. out projection folded into the value matrix akp = akbd.T @ Wp.T
     (+ bp/8 via a rank-1 matmul, since softmax rows sum to 1 per head).

v4 performance notes (v1 198us, v2 181us, v3 180us):
  - tokens are PERMUTED on the host into pooling-block-major order
    ((hi,wi,hr,wr) instead of raster), so each 14x14 pooling block is
    196 CONTIGUOUS columns: pooling becomes ONE stride-1 TensorReduce
    per (group, ci) slab and is eligible for the DVE 16-bit fast path.
    Attention is token-order agnostic; the host un-permutes the output.
  - x transposed on the HOST into [b, ci, 128, n'] bf16 slabs; loads
    split across BOTH HWDGE rings (one ring sustains only ~233 GB/s).
  - output produced TRANSPOSED in bf16 (3 matmuls/chunk), buffered per
    batch in SBUF, stored as 3 contiguous DMAs per batch.
  - ALL chunk-loop matmul operands bf16: fp32/f32r moving operands run
    2-3x slower; the fp32 softmax-broadcast matmul was compiler-split
    into a ~2us LOW_HIGH pair per chunk (reciprocals now cast to bf16
    on ACT before the broadcast matmul).
  - elementwise work (exp, reciprocal, normalize-mult, PSUM->SBUF bf16
    casts) is spread across DVE / ACT / GPSIMD.
  - tile_wait_until floors on injected agent groups keep the greedy
    Tile scheduler from front-running big pool reduces into DVE idle
    gaps ahead of the critical agent chain.
"""

import numpy as np
import ml_dtypes
from contextlib import ExitStack, nullcontext

import concourse.bass as bass
import concourse.bacc as bacc
import concourse.mybir as mybir
import concourse.tile as tile
from concourse.bass_utils import run_bass_kernel_spmd

NCORES = 8
B_FULL = 32
BPC = B_FULL // NCORES   # 4 batches per core
N = 3136                 # 56*56
C = 384
P = 128
CH = C // P              # 3 c-chunks
NCHUNK = 448
NCH = N // NCHUNK        # 7 n-chunks per batch
HEADS = 8
D = C // HEADS           # 48
A = 16                   # agents
HA = HEADS * A           # 128
SCALE = float(D) ** -0.5
POOLN = 196.0            # 14*14 elements per pooling block

F32 = mybir.dt.float32
BF16 = mybir.dt.bfloat16

ADD = mybir.AluOpType.add
MULT = mybir.AluOpType.mult

# constant blob column layouts (per 128-partition row, in elements).
# blob16a holds only what the batch-0 agent chain needs first, so it
# can load before batch 0's x slab on the ACT ring; blob16b follows.
_F32_SECTS = [("bpv", CH), ("bdmask", CH * HA)]
_B16A_SECTS = [("wqt16", CH * C), ("wkvt16", CH * C), ("wg16", CH),
               ("ones16", HA)]
_B16B_SECTS = [("wq16", CH * C), ("wpt16", CH * C),
               ("hsel40", 40), ("hselt16", HA), ("bp816", C)]
F32_COLS = sum(n for _, n in _F32_SECTS)
B16A_COLS = sum(n for _, n in _B16A_SECTS)
B16B_COLS = sum(n for _, n in _B16B_SECTS)


def _offsets(sects):
    out, o = {}, 0
    for name, n in sects:
        out[name] = (o, n)
        o += n
    return out


F32_OFF = _offsets(_F32_SECTS)
B16A_OFF = _offsets(_B16A_SECTS)
B16B_OFF = _offsets(_B16B_SECTS)

# agent-phase batch groups + scheduler floors (ms of simulated time;
# keeps each group's pool reduces out of the DVE queue until its x
# slabs have landed).
GROUPS = [[0], [1], [2], [3]]
G_FLOOR = [None, 0.024, 0.034, 0.044]


def build_nc(stage=None):
    nc = bacc.Bacc(None, target_bir_lowering=False, debug=False)

    x16t = nc.dram_tensor("x16t", [BPC, CH, P, N], BF16,
                          kind="ExternalInput")
    blob16a = nc.dram_tensor("blob16a", [P, B16A_COLS], BF16,
                             kind="ExternalInput")
    blob16b = nc.dram_tensor("blob16b", [P, B16B_COLS], BF16,
                             kind="ExternalInput")
    blobf = nc.dram_tensor("blobf", [P, F32_COLS], F32, kind="ExternalInput")
    out16t = nc.dram_tensor("out16t", [BPC, CH, P, N], BF16,
                            kind="ExternalOutput")

    with tile.TileContext(nc) as tc, ExitStack() as ctx:
        consts = ctx.enter_context(tc.tile_pool(name="consts", bufs=1))
        xtp = ctx.enter_context(tc.tile_pool(name="xt", bufs=1))
        agents = ctx.enter_context(tc.tile_pool(name="agents", bufs=3))
        akpp = ctx.enter_context(tc.tile_pool(name="akpp", bufs=3))
        chunkp = ctx.enter_context(tc.tile_pool(name="chunk", bufs=4))
        obuf = ctx.enter_context(tc.tile_pool(name="obuf", bufs=2))
        psA = ctx.enter_context(
            tc.tile_pool(name="psA", bufs=1, space=bass.MemorySpace.PSUM))
        psL = ctx.enter_context(
            tc.tile_pool(name="psL", bufs=2, space=bass.MemorySpace.PSUM))
        psS = ctx.enter_context(
            tc.tile_pool(name="psS", bufs=1, space=bass.MemorySpace.PSUM))
        psB = ctx.enter_context(
            tc.tile_pool(name="psB", bufs=1, space=bass.MemorySpace.PSUM))
        psO = ctx.enter_context(
            tc.tile_pool(name="psO", bufs=3, space=bass.MemorySpace.PSUM))

        # ---- x loads split across both HWDGE rings (b0 slabs first on
        # each so batch 0 completes ~7us in). SP: c0+c1 slabs; ACT: c2
        # slabs interleaved with the const blobs.
        xt_all = xtp.tile([P, CH, BPC * N], BF16, tag="xt")
        sb16a = consts.tile([P, B16A_COLS], BF16, tag="sb16a")
        sb16b = consts.tile([P, B16B_COLS], BF16, tag="sb16b")
        sbf = consts.tile([P, F32_COLS], F32, tag="sbf")

        def ld(b, ci, eng):
            eng.dma_start(xt_all[:, ci, b * N:(b + 1) * N], x16t[b, ci])

        # batch 0's three slabs split in half across both rings so each
        # lands as early as possible (each ring sustains ~160 GB/s when
        # both are active)
        nc.scalar.dma_start(sb16a[:], blob16a[:])
        H2 = N // 2
        for ci in range(CH):
            nc.sync.dma_start(xt_all[:, ci, 0:H2], x16t[0, ci, :, 0:H2])
            nc.scalar.dma_start(xt_all[:, ci, H2:N], x16t[0, ci, :, H2:N])
        nc.scalar.dma_start(sb16b[:], blob16b[:])
        nc.scalar.dma_start(sbf[:], blobf[:])
        ld(1, 0, nc.sync)
        ld(1, 1, nc.sync)
        ld(1, 2, nc.scalar)
        ld(2, 0, nc.sync)
        ld(2, 1, nc.sync)
        ld(2, 2, nc.scalar)
        ld(3, 0, nc.sync)
        ld(3, 1, nc.sync)
        ld(3, 2, nc.scalar)

        def fview(name, nmid):
            o, n = F32_OFF[name]
            v = sbf[:, o:o + n]
            return v.rearrange("p (a b) -> p a b", a=nmid) if nmid else v

        def bview(name, nmid):
            if name in B16A_OFF:
                o, n = B16A_OFF[name]
                v = sb16a[:, o:o + n]
            else:
                o, n = B16B_OFF[name]
                v = sb16b[:, o:o + n]
            return v.rearrange("p (a b) -> p a b", a=nmid) if nmid else v

        wq_sb = bview("wq16", CH)
        wqt_sb = bview("wqt16", CH)
        wkvt_sb = bview("wkvt16", CH)
        wpt16_sb = bview("wpt16", CH)
        wg_sb = bview("wg16", CH)
        ones16_sb = bview("ones16", 0)[0:1, :]
        hsel40_sb = bview("hsel40", 0)
        hsel16_sb = hsel40_sb[:, 0:HEADS]
        hselt16_v = bview("hselt16", 0)
        hselt16_sb = [hselt16_v[0:HEADS, :],
                      hselt16_v[32:32 + HEADS, :]]
        bp816_sb = bview("bp816", 0)[0:1, :]
        bp_sb = fview("bpv", CH)
        bdm_sb = fview("bdmask", CH)

        bstate = {}

        def agent_steps(bs):
            """Agent phase for a group of batches (pool -> projections ->
            gating -> agent_new -> mbdT/akp), batched along the free axis
            so the many small matmuls get L*16 columns instead of 16."""
            L = len(bs)
            k0 = bs[0]
            AG = L * A
            st = {}

            def pool(ci):
                def f():
                    if ci == 0:
                        st["xpT"] = agents.tile([P, CH, AG], BF16,
                                                tag="xpT", name="xpT")
                    # block-major token order: contiguous stride-1
                    # reduces, split into 4-agent pieces (~0.9us) so
                    # they never head-of-line-block the DVE queue
                    v = xt_all[:, ci, k0 * N:(k0 + L) * N].rearrange(
                        "p (a r) -> p a r", a=L * A)
                    xp = st["xpT"][:, ci, :].rearrange(
                        "p (a x) -> p a x", a=L * A)
                    with nc.allow_low_precision(reason="bf16 pooling"):
                        for a0 in range(0, L * A, 4):
                            nc.vector.tensor_reduce(
                                xp[:, a0:a0 + 4, :], v[:, a0:a0 + 4, :],
                                axis=mybir.AxisListType.X, op=ADD)
                return f

            def proj(dst_name, w_sb):
                def f():
                    ps = psA.tile([P, CH, AG], F32, tag="ag", name="agps")
                    for co in range(CH):
                        for ci in range(CH):
                            nc.tensor.matmul(
                                ps[:, co, :], w_sb[:, ci, co * P:(co + 1) * P],
                                st["xpT"][:, ci, :],
                                start=(ci == 0), stop=(ci == CH - 1))
                    t = agents.tile([P, CH, AG], BF16, tag=dst_name,
                                    name=dst_name)
                    nc.scalar.copy(t[:], ps[:])
                    st[dst_name] = t
                return f

            def gate1():
                a_ps = psA.tile([1, AG], F32, tag="ag", name="agps")
                for ci in range(CH):
                    nc.tensor.matmul(a_ps[:], wg_sb[:, ci, :],
                                     st["aqT"][:, ci, :],
                                     start=(ci == 0), stop=(ci == CH - 1))
                a_sb = agents.tile([1, AG], BF16, tag="a_sb", name="a_sb")
                nc.scalar.copy(a_sb[:], a_ps[:])  # SCALE folded into wg16
                ar_ps = psA.tile([P, AG], F32, tag="ag", name="agps")
                nc.tensor.matmul(ar_ps[:], ones16_sb[:], a_sb[:],
                                 start=True, stop=True)
                st["ar_ps"] = ar_ps

            def gate2():
                gscr = agents.tile([P, CH, AG], F32, tag="gscr", name="gscr")
                for ci in range(CH):
                    nc.vector.tensor_mul(gscr[:, ci, :], st["aqT"][:, ci, :],
                                         st["ar_ps"][:])
                gvec = agents.tile([P, CH * L], F32, tag="gvec", name="gvec")
                nc.vector.tensor_reduce(
                    gvec[:].rearrange("p (cb x) -> p cb x", cb=CH * L),
                    gscr[:].rearrange("p c (b a) -> p (c b) a", b=L),
                    axis=mybir.AxisListType.X, op=ADD)
                gkT = agents.tile([P, CH, AG], BF16, tag="gkT", name="gkT")
                for ci in range(CH):
                    for j in range(L):
                        nc.vector.tensor_scalar_mul(
                            gkT[:, ci, j * A:(j + 1) * A],
                            st["akT"][:, ci, j * A:(j + 1) * A],
                            gvec[:, ci * L + j:ci * L + j + 1])
                st["gkT"] = gkT

            def an_mms():
                an_ps = psA.tile([P, CH, AG], F32, tag="ag", name="agps")
                for co in range(CH):
                    for ci in range(CH):
                        nc.tensor.matmul(
                            an_ps[:, co, :],
                            wpt16_sb[:, ci, co * P:(co + 1) * P],
                            st["gkT"][:, ci, :],
                            start=(ci == 0), stop=(ci == CH - 1))
                st["an_ps"] = an_ps

            def bd():
                an_st = agents.tile([P, CH, AG], F32, tag="an_st",
                                    name="an_st")
                for ci in range(CH):
                    nc.vector.tensor_add(an_st[:, ci, :],
                                         st["an_ps"][:, ci, :],
                                         st["aqT"][:, ci, :])
                    nc.vector.tensor_scalar(an_st[:, ci, :], an_st[:, ci, :],
                                            bp_sb[:, ci, :], SCALE, ADD, MULT)
                anbd = agents.tile([P, CH, L * HA], BF16, tag="anbd",
                                   name="anbd")
                akbd = agents.tile([P, CH, L * HA], BF16, tag="akbd",
                                   name="akbd")
                with nc.allow_low_precision(reason="bf16 blockdiag"):
                    for ci in range(CH):
                        src_an = an_st[:, ci, :].rearrange(
                            "p (b a) -> p b a", b=L).unsqueeze(2).broadcast_to(
                            [P, L, HEADS, A])
                        src_ak = st["akT"][:, ci, :].rearrange(
                            "p (b a) -> p b a", b=L).unsqueeze(2).broadcast_to(
                            [P, L, HEADS, A])
                        msk = bdm_sb[:, ci, :].rearrange(
                            "p (h a) -> p h a",
                            h=HEADS).unsqueeze(1).broadcast_to(
                            [P, L, HEADS, A])
                        nc.vector.tensor_mul(
                            anbd[:, ci, :].rearrange("p (b h a) -> p b h a",
                                                     b=L, h=HEADS),
                            src_an, msk)
                        nc.vector.tensor_mul(
                            akbd[:, ci, :].rearrange("p (b h a) -> p b h a",
                                                     b=L, h=HEADS),
                            src_ak, msk)
                st["anbd"], st["akbd"] = anbd, akbd

            def mbd():
                mbdT = agents.tile([P, CH, L * HA], BF16, tag="mbdT",
                                   name="mbdT")
                for ci in range(CH):
                    mbd_ps = psA.tile([P, L * HA], F32, tag="ag",
                                      name="mbd_ps")
                    for co in range(CH):
                        nc.tensor.matmul(mbd_ps[:],
                                         wq_sb[:, co, ci * P:(ci + 1) * P],
                                         st["anbd"][:, co, :],
                                         start=(co == 0), stop=(co == CH - 1))
                    nc.vector.tensor_copy(mbdT[:, ci, :], mbd_ps[:])
                st["mbdT"] = mbdT
                for j, b in enumerate(bs):
                    bstate[b] = {"mbdT": mbdT, "j": j}

            def akp(j, b):
                def f():
                    akp_ps = psA.tile([P, C], F32, tag="ag", name="akp_ps")
                    for ci in range(CH):
                        nc.tensor.matmul(
                            akp_ps[:],
                            st["akbd"][:, ci, j * HA:(j + 1) * HA],
                            wpt16_sb[:, ci, :], start=(ci == 0), stop=False)
                    nc.tensor.matmul(akp_ps[:], ones16_sb[:], bp816_sb[:],
                                     start=False, stop=True)
                    akp_sb = akpp.tile([P, C], BF16, tag="akp",
                                       name="akp_sb")
                    nc.scalar.copy(akp_sb[:], akp_ps[:])
                    bstate[b]["akp"] = akp_sb
                return f

            steps = [pool(0), pool(1), pool(2),
                     proj("aqT", wqt_sb), proj("akT", wkvt_sb),
                     gate1, gate2, an_mms, bd, mbd]
            steps += [akp(j, b) for j, b in enumerate(bs)]
            return steps

        NSLOT = BPC * NCH
        slots = [(b, t) for b in range(BPC) for t in range(NCH)]
        sstate = [dict() for _ in range(NSLOT)]

        def s0(i):
            b, t = slots[i]
            st = sstate[i]
            l_ps = psL.tile([P, NCHUNK], F32, tag="l", name="l_ps")
            mbdT, j = bstate[b]["mbdT"], bstate[b]["j"]
            for ci in range(CH):
                nc.tensor.matmul(l_ps[:],
                                 mbdT[:, ci, j * HA:(j + 1) * HA],
                                 xt_all[:, ci, b * N + t * NCHUNK:
                                        b * N + (t + 1) * NCHUNK],
                                 start=(ci == 0), stop=(ci == CH - 1))
            expT = chunkp.tile([P, NCHUNK], BF16, tag="exp", name="expT")
            with nc.allow_low_precision(reason="bf16 attn"):
                nc.scalar.activation(expT[:], l_ps[:],
                                     mybir.ActivationFunctionType.Exp)
            st["expT"] = expT

        def s1pair(p):
            # two chunks share one reciprocal + one bf16 cast: the two
            # sums matmuls write disjoint partition slices of one PSUM
            # tile, halving the per-chunk DVE/ACT normalization cost
            ii = (2 * p, 2 * p + 1)
            # matmul out base partition must be 0/32/64: pack the pair's
            # sums at partitions 0-7 and 32-39 of one PSUM tile
            s_ps = psS.tile([40, NCHUNK], F32, tag="s", name="s_ps")
            # first MM writes all 40 partitions (cols 8-39 of hsel40 are
            # zero) so the full-tile reciprocal never reads uninit PSUM
            nc.tensor.matmul(s_ps[:], hsel40_sb[:],
                             sstate[ii[0]]["expT"][:],
                             start=True, stop=True)
            nc.tensor.matmul(s_ps[32:32 + HEADS, :], hsel16_sb[:],
                             sstate[ii[1]]["expT"][:],
                             start=True, stop=True)
            rec = chunkp.tile([40, NCHUNK], F32, tag="rec", name="rec")
            nc.vector.reciprocal_approx_fast(rec[:], s_ps[:])
            rec16 = chunkp.tile([40, NCHUNK], BF16, tag="rec16",
                                name="rec16")
            nc.scalar.copy(rec16[:], rec[:])
            for k, i in enumerate(ii):
                st = sstate[i]
                b_ps = psB.tile([P, NCHUNK], F32, tag="b", name="b_ps")
                nc.tensor.matmul(b_ps[:],
                                 hselt16_sb[k],
                                 rec16[32 * k:32 * k + HEADS, :],
                                 start=True, stop=True)
                attnT = chunkp.tile([P, NCHUNK], BF16, tag="attn",
                                    name="attnT")
                with nc.allow_low_precision(reason="bf16 attn"):
                    nc.vector.tensor_mul(attnT[:], st["expT"][:], b_ps[:])
                st["attnT"] = attnT

        def s2(i):
            b, t = slots[i]
            st = sstate[i]
            attnT = st["attnT"]
            akp_sb = bstate[b]["akp"]
            if t == 0:
                bstate[b]["ob"] = obuf.tile([P, CH, N], BF16, tag="ob",
                                            name="ob")
            ob = bstate[b]["ob"]
            ns = slice(t * NCHUNK, (t + 1) * NCHUNK)
            for ci in range(CH):
                o_ps = psO.tile([P, NCHUNK], F32, tag="op", name="o_ps")
                nc.tensor.matmul(o_ps[:], akp_sb[:, ci * P:(ci + 1) * P],
                                 attnT[:], start=True, stop=True)
                if ci == 0 or (ci == 2 and i % 2 == 0):
                    nc.vector.tensor_copy(ob[:, ci, ns], o_ps[:])
                else:
                    nc.scalar.copy(ob[:, ci, ns], o_ps[:])
            # batches 0-2: one big store per (batch, ci); final batch:
            # store per pair of chunks so the tail isn't one 2.4MB store
            last = b == BPC - 1
            if (not last and t == NCH - 1) or (
                    last and (t % 2 == 1 or t == NCH - 1)):
                if last:
                    c0 = (t - 1) * NCHUNK if t % 2 == 1 else t * NCHUNK
                else:
                    c0 = 0
                cs = slice(c0, (t + 1) * NCHUNK)
                for ci in range(CH):
                    eng = nc.sync if ci == 0 else nc.scalar
                    eng.dma_start(out16t[b, ci, :, cs], ob[:, ci, cs])

        # prologue: batch 0's agent chain runs exposed.
        for f in agent_steps(GROUPS[0]):
            f()
        # inject group g's agent steps during earlier batches' chunk
        # slots, with scheduler floors so they can't run before their
        # x slabs land.
        inject = {}
        for g, (bhost, t0, t1) in ((1, (0, 1, 7)), (2, (1, 0, 7)),
                                   (3, (2, 0, 7))):
            steps = agent_steps(GROUPS[g])
            floor = G_FLOOR[g]
            nsl = t1 - t0
            for t in range(t0, t1):
                k0 = len(steps) * (t - t0) // nsl
                k1 = len(steps) * (t - t0 + 1) // nsl
                inject.setdefault((bhost, t), []).extend(
                    (floor, f) for f in steps[k0:k1])

        NP = NSLOT // 2
        for p in range(NP + 2):
            for i in (2 * p, 2 * p + 1):
                if i < NSLOT:
                    s0(i)
                    for floor, f in inject.get(slots[i], ()):
                        cmgr = (tc.tile_wait_until(floor)
                                if floor is not None else nullcontext())
                        with cmgr:
                            f()
            if 1 <= p <= NP:
                s1pair(p - 1)
            if p >= 2:
                s2(2 * p - 4)
                s2(2 * p - 3)

    nc.finalize()
    return nc


_CACHE = {}


def _get_nc():
    if "nc" not in _CACHE:
        _CACHE["nc"] = build_nc()
    return _CACHE["nc"]


def _make_const_inputs(Wq, Wkv, w_g, Wp, bp):
    Wq = np.ascontiguousarray(np.asarray(Wq, np.float32))
    Wkv = np.ascontiguousarray(np.asarray(Wkv, np.float32))
    Wp = np.ascontiguousarray(np.asarray(Wp, np.float32))
    w_g = np.ascontiguousarray(np.asarray(w_g, np.float32))
    bp = np.ascontiguousarray(np.asarray(bp, np.float32))

    def chunked(m):  # [CH*P, X] -> [P, CH*X] partition-major layout
        x = m.reshape(CH, P, -1).transpose(1, 0, 2)
        return np.ascontiguousarray(x).reshape(P, -1)

    hsel = np.zeros((HA, HEADS), np.float32)
    hsel[np.arange(HA), np.arange(HA) // A] = 1.0
    bdmask = np.zeros((C, HA), np.float32)
    for c in range(C):
        h = c // D
        bdmask[c, h * A:(h + 1) * A] = 1.0

    fparts = {
        "bpv": chunked(bp.reshape(C, 1)),
        "bdmask": chunked(bdmask),
    }
    blobf = np.concatenate([fparts[n] for n, _ in _F32_SECTS], axis=1)

    z = np.zeros((P, HA), np.float32)
    z[0] = 1.0
    zh = np.zeros((P, HA), np.float32)
    zh[:HEADS] = hsel.T
    zh[32:32 + HEADS] = hsel.T
    zb = np.zeros((P, C), np.float32)
    zb[0] = bp / HEADS
    bparts = {
        "wq16": chunked(Wq),
        "wqt16": chunked(Wq.T / POOLN),
        "wkvt16": chunked(Wkv.T / POOLN),
        "wpt16": chunked(Wp.T),
        "wg16": chunked(w_g * SCALE),
        "ones16": z,
        "hsel40": np.concatenate(
            [hsel, np.zeros((HA, 32), np.float32)], axis=1),
        "hselt16": zh,
        "bp816": zb,
    }
    blob16a = np.concatenate(
        [bparts[n] for n, _ in _B16A_SECTS],
        axis=1).astype(ml_dtypes.bfloat16)
    blob16b = np.concatenate(
        [bparts[n] for n, _ in _B16B_SECTS],
        axis=1).astype(ml_dtypes.bfloat16)
    return {"blob16a": blob16a, "blob16b": blob16b, "blobf": blobf}


def _permute_tokens(x):
    # raster (56x56) -> pooling-block-major (hi, wi, hr, wr)
    b = x.shape[0]
    return x.reshape(b, 4, 14, 4, 14, C).transpose(
        0, 1, 3, 2, 4, 5).reshape(b, N, C)


def _unpermute_tokens(y):
    # pooling-block-major -> raster
    b = y.shape[0]
    return y.reshape(b, 4, 4, 14, 14, C).transpose(
        0, 1, 3, 2, 4, 5).reshape(b, N, C)


def kernel(x, H=56, W=56, Wq=None, Wkv=None, w_g=None, Wp=None, bp=None,
           _trace=False, _trace_kwargs=None):
    x = np.asarray(x)
    assert x.shape == (B_FULL, N, C), x.shape
    x16 = np.asarray(x, np.float32).astype(ml_dtypes.bfloat16)
    # host-side: permute tokens to block-major, then transpose to
    # [b, ci, p, n'] slabs (plain DMAs on device)
    x16p = _permute_tokens(x16)
    x16t = np.ascontiguousarray(
        x16p.reshape(B_FULL, N, CH, P).transpose(0, 2, 3, 1))

    consts = _make_const_inputs(Wq, Wkv, w_g, Wp, bp)
    in_maps = []
    for c in range(NCORES):
        m = dict(consts)
        m["x16t"] = np.ascontiguousarray(x16t[c * BPC:(c + 1) * BPC])
        in_maps.append(m)

    nc = _get_nc()
    res = run_bass_kernel_spmd(nc, in_maps, list(range(NCORES)),
                               trace=_trace, **(_trace_kwargs or {}))
    # un-transpose + un-permute + upcast on host
    outs = np.concatenate(
        [_unpermute_tokens(
            np.asarray(res.results[c]["out16t"]).astype(np.float32)
            .transpose(0, 3, 1, 2).reshape(BPC, N, C))
         for c in range(NCORES)], axis=0)
    if _trace:
        return outs, res
    return outs
